# revision 41
# baseline (speedup 1.0000x reference)
"""ANFIS forward kernel for Trainium2 (8 NeuronCores, data-parallel).

out[b] = sum_r(w_r * clip(conseq_r, 0, 100)) / max(sum_r w_r, 1e-8)
w_r = prod_{j in rule r} exp(-0.5 * ((x[dim(j)] - c_j) / sigma_j)^2)

Sharding: pure data-parallel, batch split 8 ways (262144 rows per core),
no collectives.  The 11+11+30 scalar parameters are folded on the host
into instruction immediates plus one tiny [128, 96] constant weight
tensor, so the device kernel's inputs are the x shard and that constant.

Algorithm (per core).  Each rule picks exactly one membership function
per input dim, so with E_ab = mu_a * mu_b over the 9 (dim0, dim1) MF
pairs and F_cd = mu_c * mu_d over the 6 (dim2, dim3) pairs, every rule
weight is w_r = E_{ab(r)} * F_{cd(r)} and

  num = sum_cd F_cd * G'_cd   with  G' = C_hat^T E   (C_hat[ab, cd] = c_r)
  den = sum_cd F_cd * G_cd    with  G  = D_hat^T E   (0/1 rule mask)

Device pipeline (batch-major on 128 partitions, bf16 after the exp):
  stage A (ACT): v_j = (scale_j*x_d + bias_j)^2 fused-affine Squares,
     then one batched Exp -> mu [128, 11, m]
  stage B per 512-column chunk:
   - DVE/Pool: E products into a 16-slot interleaved tile, F dense
   - one blocked DMA-xbar transpose (sync queue) -> feature-major
     [128 = 16 elements x (9 E + 7 pad) slots, columns]
   - PE: single stationary block-diagonal matmul [128, 96] computing all
     12 G/G' values for 8 elements per streamed column (fp32 PSUM)
   - DVE/ACT: PSUM -> SBUF bf16 copy; DMA-xbar transpose back to
     batch-major; 12 products + adds; out = num * recip(max(den, 1e-8))

Both xbar transposes run on the same HWDGE queue: concurrent transposes
from two queues cross-wire the xbar (observed on HW).
"""

import sys

if "/opt/trn_rl_repo" not in sys.path:
    sys.path.insert(0, "/opt/trn_rl_repo")

import numpy as np

# ---------------------------------------------------------------------------
# Problem constants (hardcoded from the problem spec)
# ---------------------------------------------------------------------------
BATCH = 2097152
N_CORES = 8
B_LOC = BATCH // N_CORES  # 262144 rows per core

RULE_ANTECEDENTS = np.array(
    [(0, 3, 8, 9), (0, 3, 8, 10), (0, 3, 7, 9), (0, 4, 8, 9), (0, 3, 7, 10),
     (0, 4, 8, 10), (1, 3, 8, 9), (0, 5, 8, 9), (1, 3, 8, 10), (0, 3, 6, 9),
     (0, 4, 7, 10), (1, 4, 7, 9), (1, 3, 7, 10), (1, 3, 6, 9), (0, 5, 7, 10),
     (0, 4, 6, 9), (1, 4, 8, 10), (1, 3, 7, 9), (2, 3, 7, 9), (0, 5, 6, 9),
     (1, 5, 7, 9), (2, 4, 8, 9), (2, 5, 6, 10), (2, 5, 6, 9), (2, 4, 6, 10),
     (1, 5, 6, 10), (2, 5, 7, 10), (1, 5, 6, 9), (2, 4, 6, 9), (1, 4, 6, 10)],
    dtype=np.int32,
)  # [30, 4]
DIM_MAP = np.array([0, 0, 0, 1, 1, 1, 2, 2, 2, 3, 3], dtype=np.int32)

# Pair factorization: rule r = (a, b, c, d) -> E index a*3+(b-3), F index
# (c-6)*2+(d-9).  All 9 AB pairs and all 6 CD pairs occur.
AB_OF_RULE = [int(r[0]) * 3 + (int(r[1]) - 3) for r in RULE_ANTECEDENTS]
CD_OF_RULE = [(int(r[2]) - 6) * 2 + (int(r[3]) - 9) for r in RULE_ANTECEDENTS]

_COMPILED = {}

# Pair/rule factorization: w_r = E_{ab(r)} * F_{cd(r)};
# num = sum_cd F_cd * (C_hat^T E)_cd, den = sum_cd F_cd * (D_hat^T E)_cd.
CHAT = np.zeros((9, 6))
DHAT = np.zeros((9, 6))


def _fill_chat(cr):
    ch = np.zeros((9, 6))
    dh = np.zeros((9, 6))
    for r in range(30):
        ch[AB_OF_RULE[r], CD_OF_RULE[r]] += cr[r]
        dh[AB_OF_RULE[r], CD_OF_RULE[r]] += 1.0
    return ch, dh


def _build_lhsT_g(cr):
    """Stationary PE weights [128, 96] f16.

    Rows (e, s) = 16*e + s for subgroup e in 0..7, slot s in 0..15
    (slots 0..8 hold E_ab, 9..15 are zero padding).
    Cols 8*l + e for output l in 0..11 (l<6: num-side C_hat, else D_hat).
    """
    ch, dh = _fill_chat(cr)
    W = np.zeros((16, 12))
    W[0:9, 0:6] = ch
    W[0:9, 6:12] = dh
    import ml_dtypes
    lhsT = np.zeros((128, 96), ml_dtypes.bfloat16)
    for e in range(8):
        for s in range(16):
            for l in range(12):
                lhsT[16 * e + s, 8 * l + e] = ml_dtypes.bfloat16(W[s, l])
    return lhsT


def _build2(scales, biases, cr, rep=1, M=512, nbufs=2):
    """Phase-2 kernel: PE-based rule reduction via bf16 DMA transposes.

    Stage A (per shard): mu_j = exp(-(scale_j*x_d+bias_j)^2) at large free
    dim (ACT overhead amortized).  Stage B (per M-chunk): pair products,
    blocked DMA transpose to feature-major, one stationary block-diagonal
    matmul producing the 12 per-element linear maps G'/G, DMA transpose
    back, cheap batch-major dots and the normalized division.
    """
    import concourse.bass as bass  # noqa: F401
    import concourse.tile as tile
    from concourse import bacc, mybir

    f32 = mybir.dt.float32
    f16 = mybir.dt.bfloat16
    Square = mybir.ActivationFunctionType.Square
    Exp = mybir.ActivationFunctionType.Exp

    P = 128
    ROWS_PER_PART = B_LOC // P          # 2048
    NCHUNK = ROWS_PER_PART // M
    NSLOT = 16
    NCOLS = P * M // 8   # columns per chunk (8 elements per column)

    nc = bacc.Bacc("TRN2", target_bir_lowering=False, debug=False,
                   num_devices=N_CORES)
    x_ap = nc.dram_tensor("x", [B_LOC, 4], f32, kind="ExternalInput").ap()
    gw_ap = nc.dram_tensor("gw", [128, 96], f16, kind="ExternalInput").ap()
    out_ap = nc.dram_tensor("out", [B_LOC, 1], f32, kind="ExternalOutput").ap()

    MA = 1024                            # stage-A free dim per op
    NCA = ROWS_PER_PART // MA            # 2
    xa_view = x_ap.rearrange("(p n m) d -> p n (m d)", p=P, m=MA)
    out_view = out_ap.rearrange("(p n m) o -> p n (m o)", p=P, m=M)

    MF_GROUPS = [(0, 4), (4, 4), (8, 3)]
    E_PAIRS = [(a, 3 + b) for a in range(3) for b in range(3)]
    F_PAIRS = [(6 + c, 9 + d) for c in range(3) for d in range(2)]

    with tile.TileContext(nc) as tc:
        with (
            tc.tile_pool(name="singles", bufs=1) as singles,
            tc.tile_pool(name="mu", bufs=2) as mu_pool,
            tc.tile_pool(name="ff", bufs=nbufs) as f_pool,
            tc.tile_pool(name="ft", bufs=2) as ft_pool,
            tc.tile_pool(name="gsb", bufs=2) as gsb_pool,
            tc.tile_pool(name="gb", bufs=2) as gb_pool,
            tc.tile_pool(name="acc", bufs=2) as acc_pool,
            tc.tile_pool(name="fin", bufs=nbufs) as fin_pool,
            tc.tile_pool(name="psg", bufs=nbufs, space="PSUM") as psg_pool,
        ):
            bias_t = singles.tile([P, 11], f32)
            for j in range(11):
                nc.vector.memset(bias_t[:, j:j + 1], float(biases[j]))
            lhsT_g = singles.tile([128, 96], f16)
            nc.sync.dma_start(lhsT_g[:, :], gw_ap[:, :])
            ef_t = singles.tile([P, M, NSLOT], f16)
            nc.vector.memset(ef_t[:, :, 9:16], 0.0)

            for _rep in range(rep):
                # ---- stage A: mu per half at large free dim ----
                mu_halves = []
                with (tc.tile_pool(name="xa", bufs=1) as xa_pool,
                      tc.tile_pool(name="va", bufs=1) as va_pool):
                    for n in range(NCA):
                        mu = mu_pool.tile([P, 11, MA], f16, tag="mu")
                        mu_halves.append(mu)
                        x_sb = xa_pool.tile([P, MA, 4], f32)
                        nc.scalar.dma_start(
                            x_sb[:, :, :].rearrange("p m d -> p (m d)"),
                            xa_view[:, n, :])
                        for (g0, glen) in MF_GROUPS:
                            vg = va_pool.tile([P, 4, MA], f32, tag="va")
                            for jj in range(glen):
                                j = g0 + jj
                                d = int(DIM_MAP[j])
                                nc.scalar.activation(
                                    vg[:, jj, :], x_sb[:, :, d], Square,
                                    bias=bias_t[:, j:j + 1],
                                    scale=float(scales[j]))
                            nc.scalar.activation(
                                mu[:, g0:g0 + glen, :],
                                vg[:, 0:glen, :], Exp, scale=-1.0)

                # ---- stage B: per-chunk products, PE reduction, finals ----
                def emit_front(i):
                    mu = mu_halves[(i * M) // MA]
                    msl = slice((i * M) % MA, (i * M) % MA + M)
                    # E products into interleaved slots (strided dest, 1x)
                    for idx, (a, b) in enumerate(E_PAIRS):
                        eng = nc.gpsimd if idx >= 4 else nc.vector
                        eng.tensor_mul(ef_t[:, :, idx], mu[:, a, msl],
                                       mu[:, b, msl])
                    # F products dense (2x)
                    f_t = f_pool.tile([P, 6, M], f16)
                    for idx, (a, b) in enumerate(F_PAIRS):
                        nc.vector.tensor_mul(f_t[:, idx, :], mu[:, a, msl],
                                             mu[:, b, msl])

                    fT = ft_pool.tile([128, NSLOT * M // 128, 128], f16)
                    NJH = NSLOT * M // 128 // 2
                    efv = ef_t[:, :, :].rearrange("p q s -> p (q s)")
                    for th in range(2):
                        nc.sync.dma_start_transpose(
                            fT[:, th * NJH:(th + 1) * NJH, :],
                            efv[:, th * NJH * 128:(th + 1) * NJH * 128])
                    return f_t, fT

                def emit_back(i, f_t, fT):
                    nacc = acc_pool.tile([P, 6, M], f16, tag="nacc")
                    dacc = acc_pool.tile([P, 6, M], f16, tag="dacc")
                    HC = NCOLS // 2          # columns per half
                    HJ = HC // 128           # j-blocks per half
                    HM = M // 2              # elements/partition per half
                    for h in range(2):
                        gsb = gsb_pool.tile([96, HC], f16)
                        for piece in range(HC // 2048):
                            psg = psg_pool.tile([96, 2048], f32)
                            for k in range(4):
                                jb = h * (NCOLS // 2048 // 2) * 16 \
                                    + piece * 16 + 4 * k
                                nc.tensor.matmul(
                                    psg[:, 512 * k:512 * (k + 1)],
                                    lhsT_g[:, :], fT[:, jb:jb + 4, :],
                                    start=True, stop=True)
                            eng = nc.vector if (h + piece) % 2 == 0 \
                                else nc.scalar
                            if eng is nc.vector:
                                eng.tensor_copy(
                                    gsb[:, piece * 2048:(piece + 1) * 2048],
                                    psg[:, :])
                            else:
                                eng.copy(
                                    gsb[:, piece * 2048:(piece + 1) * 2048],
                                    psg[:, :])
                        gb = gb_pool.tile([128, HJ, 96], f16)
                        nc.sync.dma_start_transpose(gb[:, :, :], gsb[:, :])

                        hsl = slice(h * HM, (h + 1) * HM)
                        for l in range(6):
                            fv = f_t[:, l, hsl].rearrange(
                                "p (j e) -> p j e", e=8)
                            nc.vector.tensor_mul(
                                nacc[:, l, hsl].rearrange(
                                    "p (j e) -> p j e", e=8),
                                fv, gb[:, :, 8 * l:8 * l + 8])
                            nc.gpsimd.tensor_mul(
                                dacc[:, l, hsl].rearrange(
                                    "p (j e) -> p j e", e=8),
                                fv, gb[:, :, 8 * (l + 6):8 * (l + 6) + 8])
                    for t, eng in ((nacc, nc.vector), (dacc, nc.vector)):
                        eng.tensor_add(t[:, 0:3, :], t[:, 0:3, :],
                                       t[:, 3:6, :])
                        eng.tensor_add(t[:, 0:1, :], t[:, 0:1, :],
                                       t[:, 2:3, :])
                        eng.tensor_add(t[:, 0:1, :], t[:, 0:1, :],
                                       t[:, 1:2, :])

                    den32 = fin_pool.tile([P, M], f32, tag="den32")
                    nc.vector.tensor_scalar_max(den32[:, :], dacc[:, 0, :],
                                                1e-8)
                    nc.vector.reciprocal(den32[:, :], den32[:, :])
                    o = fin_pool.tile([P, M], f32, tag="o")
                    nc.vector.tensor_mul(o[:, :], nacc[:, 0, :], den32[:, :])
                    nc.scalar.dma_start(out_view[:, i, :], o[:, :])

                for i in range(NCHUNK):
                    emit_back(i, *emit_front(i))

    nc.compile()
    return nc


def kernel(x, c, log_s, conseq):
    from concourse.bass_utils import run_bass_kernel_spmd

    x = np.ascontiguousarray(x, dtype=np.float32)
    c = np.asarray(c, dtype=np.float32)
    log_s = np.asarray(log_s, dtype=np.float32)
    conseq = np.asarray(conseq, dtype=np.float32)

    # host-side parameter folding
    sigma = np.maximum(np.exp(log_s.astype(np.float64)), 1e-3)
    scales = 1.0 / (np.sqrt(2.0) * sigma)           # [11]
    biases = -c.astype(np.float64) * scales         # [11]
    cr = np.clip(conseq.astype(np.float64), 0.0, 100.0)  # [30]

    key = (tuple(np.round(scales, 12)), tuple(np.round(biases, 12)),
           tuple(np.round(cr, 12)))
    if key not in _COMPILED:
        _COMPILED[key] = _build2(scales, biases, cr)
    nc = _COMPILED[key]
    gw = _build_lhsT_g(cr)

    shards = [x[i * B_LOC:(i + 1) * B_LOC] for i in range(N_CORES)]
    res = run_bass_kernel_spmd(nc, [{"x": s, "gw": gw} for s in shards],
                               core_ids=list(range(N_CORES)))
    out = np.concatenate([res.results[i]["out"] for i in range(N_CORES)],
                         axis=0)
    return out.astype(np.float32)


if __name__ == "__main__":
    rng = np.random.default_rng(0)
    x = rng.uniform(0, 100, (BATCH, 4)).astype(np.float32)
    c = np.array([0.75, 2.5, 3.6, 30.0, 70.0, 90.0, 0.5, 4.0, 8.5, 0.0, 1.0],
                 dtype=np.float32)
    log_s = np.log(np.array([0.55, 0.5, 0.45, 18.0, 13.0, 10.0, 0.8, 1.2, 1.5,
                             0.15, 0.15], dtype=np.float32))
    conseq = np.concatenate([np.full(10, 75.0), np.full(12, 50.0),
                             np.full(8, 20.0)]).astype(np.float32)
    out = kernel(x, c, log_s, conseq)
    print(out.shape, out.dtype, out[:8, 0])
